# revision 1
# baseline (speedup 1.0000x reference)
"""Trainium2 Bass kernel for conv-stack + NetVLAD pooling + linear head.

Pure data parallel: 32 images sharded 4-per-core across 8 NeuronCores.

Per-core pipeline (per image):
  conv1 (3->4, 3x3 SAME, 512x512) as banded f32r matmuls over (row, ci)
  contraction; relu+w-pool-sum fused into one ACT + one DVE op per tile
  -> Y1 [128=(h%32)*4+c, 16 blk, 256 w] (f32r, sum-pooled).
  conv2 (4->16) as banded f32r matmuls with pool1's h-direction folded into
  the band; tiles paired for 512-wide moving operands; relu + w-pool-sum
  -> V [128=(r%8)*16+c, 32 t, 128 w] (f32r).
  NetVLAD: logitsT = V.T @ A_blk per tile on the PE (pool2-h folded into
  A_blk, bias added on DVE), softmax over k=4 in the free dim; xf^T from a
  bf16 PE transpose of V + pair-sum on DVE; per-tile Gram via ONE bf16
  matmul with a block-padded lhsT so the q-diagonal blocks land on
  partitions {0,32,64,96}; diagonals summed on DVE; tiny normalize +
  linear finale.
"""
import sys

sys.path.insert(0, "/opt/trn_rl_repo")

import numpy as np
import concourse.bacc as bacc
import concourse.tile as tile
from concourse import mybir
from concourse.bass_utils import run_bass_kernel_spmd

F32 = mybir.dt.float32
F32R = mybir.dt.float32r
BF16 = mybir.dt.bfloat16
AX = mybir.AxisListType
ALU = mybir.AluOpType
ACTF = mybir.ActivationFunctionType

N_CORES = 8
IPC = 4  # images per core
EPS = 1e-12


def _round_f32r(a):
    u = np.ascontiguousarray(a, np.float32).view(np.uint32)
    u = (u + 0x200) & np.uint32(0xFFFFFC00)
    return u.view(np.float32)


def _build_consts(conv1_w, conv2_w, assign_w, assign_b, lin_w, lin_b):
    c1w = np.asarray(conv1_w, np.float32)
    c2w = np.asarray(conv2_w, np.float32)
    # conv1 banded lhsT, tiles re-anchored to output rows [32r-2, 32r+30):
    # rows p = a*3 + ci (a = h_in - (32r-3), 0..33), cols q = j*4 + co
    # (j = h_out - (32r-2), 0..31). variants: 0 = mid, 1 = first tile
    # (inputs h<0 and outputs h<0 masked), 2 = tail tile r=16 (only
    # outputs 510/511 and inputs 509..511 kept).
    W1 = np.zeros((102, 9, 128), np.float32)
    for dx in range(3):
        for co in range(4):
            for ci in range(3):
                for dy in range(3):
                    for j in range(32):
                        W1[(j + dy) * 3 + ci, dx, j * 4 + co] = c1w[co, ci, dy, dx]
    W1[:, 3:6, :] = W1[:, 0:3, :]
    W1[0:9, 3:6, :] = 0.0        # inputs h=-3..-1
    W1[:, 3:6, 0:8] = 0.0        # outputs h=-2,-1
    W1[:, 6:9, :] = W1[:, 0:3, :]
    W1[9:102, 6:9, :] = 0.0      # inputs h>=512
    W1[:, 6:9, 8:128] = 0.0      # outputs h>=512
    # conv2 banded lhsT with pool1-h fold (input rows are unpooled Y1 rows)
    # and the w-pools stored as SUMs: total scale 0.25. Single variant --
    # image edges are already zeros in the phase-shifted Y1 blocks.
    W2 = np.zeros((80, 3, 128), np.float32)
    for dx in range(3):
        for co in range(16):
            for ci in range(4):
                for dy in range(3):
                    for rr in range(8):
                        for half in range(2):
                            W2[(2 * rr + 2 * dy + half) * 4 + ci, dx, rr * 16 + co] = (
                                0.25 * c2w[co, ci, dy, dx]
                            )
    # logitsT rhs: rows p = rrel*16 + c, cols = q*4 + k (pool2-h fold, V sums)
    aw = np.asarray(assign_w, np.float32)
    A = np.zeros((128, 16), np.float32)
    for q in range(4):
        for k in range(4):
            for c in range(16):
                for half in range(2):
                    A[(2 * q + half) * 16 + c, q * 4 + k] = 0.25 * aw[k, c]
    brep = np.tile(np.asarray(assign_b, np.float32), 4).reshape(16)
    brep128 = np.broadcast_to(brep, (128, 16)).copy()
    return {
        "w1": _round_f32r(W1),
        "w2": _round_f32r(W2),
        "ablk": _round_f32r(A),
        "brep": brep128.astype(np.float32),
        "identb": np.eye(128, dtype=np.float32),  # bf16 transpose identity
        "cent": np.zeros(0),  # set by caller (4x centroids)
        "wlin": np.asarray(lin_w, np.float32).T.copy(),  # (64, 7)
        "linb": np.asarray(lin_b, np.float32).reshape(1, 7),
        "ones41": np.ones((4, 1), np.float32),
    }


def _build_program():
    nc = bacc.Bacc("TRN2", target_bir_lowering=False, debug=False,
                   num_devices=N_CORES)
    xin = nc.dram_tensor("x", [IPC, 3, 512, 512], F32R, kind="ExternalInput").ap()
    w1 = nc.dram_tensor("w1", [102, 9, 128], F32R, kind="ExternalInput").ap()
    w2 = nc.dram_tensor("w2", [80, 3, 128], F32R, kind="ExternalInput").ap()
    ablk = nc.dram_tensor("ablk", [128, 16], F32R, kind="ExternalInput").ap()
    brep = nc.dram_tensor("brep", [128, 16], F32, kind="ExternalInput").ap()
    identb = nc.dram_tensor("identb", [128, 128], F32, kind="ExternalInput").ap()
    cent = nc.dram_tensor("cent", [4, 16], F32, kind="ExternalInput").ap()
    wlin = nc.dram_tensor("wlin", [64, 7], F32, kind="ExternalInput").ap()
    linb = nc.dram_tensor("linb", [1, 7], F32, kind="ExternalInput").ap()
    ones41 = nc.dram_tensor("ones41", [4, 1], F32, kind="ExternalInput").ap()
    out = nc.dram_tensor("out", [IPC, 7], F32, kind="ExternalOutput").ap()

    from contextlib import ExitStack

    with tile.TileContext(nc) as tc, ExitStack() as es:
        consts = es.enter_context(tc.tile_pool(name="consts", bufs=1))
        x1p = es.enter_context(tc.tile_pool(name="x1p", bufs=3))
        y1p = es.enter_context(tc.tile_pool(name="y1p", bufs=2))
        x2p = es.enter_context(tc.tile_pool(name="x2p", bufs=3))
        vp = es.enter_context(tc.tile_pool(name="vp", bufs=2))
        xftp = es.enter_context(tc.tile_pool(name="xftp", bufs=2))
        smp = es.enter_context(tc.tile_pool(name="smp", bufs=3))
        finp = es.enter_context(tc.tile_pool(name="finp", bufs=2))
        p1p = es.enter_context(tc.tile_pool(name="p1p", bufs=2, space="PSUM"))
        p2p = es.enter_context(tc.tile_pool(name="p2p", bufs=2, space="PSUM"))
        ltp = es.enter_context(tc.tile_pool(name="ltp", bufs=2, space="PSUM"))
        vtp = es.enter_context(tc.tile_pool(name="vtp", bufs=1, space="PSUM"))
        gramp = es.enter_context(tc.tile_pool(name="gramp", bufs=1, space="PSUM"))

        w1_sb = consts.tile([102, 9, 128], F32R)
        nc.sync.dma_start(out=w1_sb, in_=w1)
        w2_sb = consts.tile([80, 3, 128], F32R)
        nc.sync.dma_start(out=w2_sb, in_=w2)
        ablk_sb = consts.tile([128, 16], F32R)
        nc.sync.dma_start(out=ablk_sb, in_=ablk)
        brep_sb = consts.tile([128, 16], F32)
        nc.sync.dma_start(out=brep_sb, in_=brep)
        identf_sb = consts.tile([128, 128], F32)
        nc.sync.dma_start(out=identf_sb, in_=identb)
        identb_sb = consts.tile([128, 128], BF16)
        nc.vector.tensor_copy(identb_sb, identf_sb)
        cent_sb = consts.tile([4, 16], F32)
        nc.sync.dma_start(out=cent_sb, in_=cent)
        wlin_sb = consts.tile([64, 7], F32)
        nc.sync.dma_start(out=wlin_sb, in_=wlin)
        linb_sb = consts.tile([1, 7], F32)
        nc.sync.dma_start(out=linb_sb, in_=linb)
        ones41_sb = consts.tile([4, 1], F32)
        nc.sync.dma_start(out=ones41_sb, in_=ones41)

        for img in range(IPC):
            # ====== conv1: 17 tiles, output rows [32r-2, 32r+30) ==========
            y1 = y1p.tile([128, 17, 258], F32R, tag="y1")
            nc.vector.memset(y1[:, :, 0:1].bitcast(F32), 0.0)
            nc.vector.memset(y1[:, :, 257:258].bitcast(F32), 0.0)
            for r in range(17):
                x1 = x1p.tile([102, 514], F32R, tag="x1")
                nc.vector.memset(x1[:, 0:1].bitcast(F32), 0.0)
                nc.vector.memset(x1[:, 513:514].bitcast(F32), 0.0)
                var1 = 1 if r == 0 else (2 if r == 16 else 0)
                a_lo = 3 if r == 0 else 0
                a_hi = 3 if r == 16 else 34
                base = 32 * r - 3
                x1v = x1.rearrange("(a c) w -> a c w", c=3)
                dmae = nc.sync if r % 2 == 0 else nc.scalar
                for ci in range(3):
                    dmae.dma_start(
                        out=x1v[a_lo:a_hi, ci, 1:513],
                        in_=xin[img, ci, base + a_lo : base + a_hi, :],
                    )
                p1 = p1p.tile([128, 512], F32, tag="p1")
                for dx in range(3):
                    nc.tensor.matmul(
                        p1, w1_sb[:, var1 * 3 + dx, :], x1[:, dx : dx + 512],
                        start=(dx == 0), stop=(dx == 2),
                    )
                p1v = p1.rearrange("p (w two) -> p w two", two=2)
                re1 = smp.tile([128, 256], F32, tag="re1")
                nc.scalar.activation(out=re1, in_=p1v[:, :, 0], func=ACTF.Relu)
                nc.vector.scalar_tensor_tensor(
                    out=y1[:, r, 1:257], in0=p1v[:, :, 1], scalar=0.0, in1=re1,
                    op0=ALU.max, op1=ALU.add,
                )

            # == conv2: even pairs read Y1 blocks directly; odd pairs are
            # == staged with 2 SBUF DMAs each (window straddles two blocks).
            v = vp.tile([128, 32, 128], F32R, tag="v")
            for pi in range(16):
                even = pi < 8
                if even:
                    b = 2 * pi            # tiles 4*pi, 4*pi+2
                    ts = (4 * pi, 4 * pi + 2)
                    rhs = y1[0:80, b : b + 2, :]
                else:
                    oi = pi - 8
                    ts = (4 * oi + 1, 4 * oi + 3)
                    x2 = x2p.tile([80, 2, 258], F32R, tag="x2")
                    for j in range(2):
                        t = ts[j]
                        b = t // 2
                        nc.scalar.dma_start(
                            out=x2[0:64, j, :], in_=y1[64:128, b, :])
                        nc.scalar.dma_start(
                            out=x2[64:80, j, :], in_=y1[0:16, b + 1, :])
                    rhs = x2[:]
                p2 = p2p.tile([128, 2, 256], F32, tag="p2")
                for dx in range(3):
                    nc.tensor.matmul(
                        p2, w2_sb[:, dx, :], rhs[:, :, dx : dx + 256],
                        start=(dx == 0), stop=(dx == 2),
                    )
                p2v = p2.rearrange("p a (w two) -> p a w two", two=2)
                for j in range(2):
                    t = ts[j]
                    re2 = smp.tile([128, 128], F32, tag="re2")
                    nc.scalar.activation(
                        out=re2, in_=p2v[:, j, :, 0], func=ACTF.Relu)
                    nc.vector.scalar_tensor_tensor(
                        out=v[:, t, :], in0=p2v[:, j, :, 1], scalar=0.0,
                        in1=re2, op0=ALU.max, op1=ALU.add,
                    )

            # ====== NetVLAD per pair of tiles (g = 0..15 over t=2g,2g+1) ==
            xft = xftp.tile([128, 32, 4, 17], BF16, tag="xft")
            nc.vector.memset(xft[:, :, :, 16:17], 1.0)
            g32 = gramp.tile([128, 68], F32, tag="gfin")
            for g in range(16):
                # bf16 copies of the two V tiles, transposed on the PE
                vt2 = vtp.tile([128, 2, 128], BF16, tag="vt2")
                lt2 = ltp.tile([128, 2, 16], F32, tag="lt2")
                for j in range(2):
                    t = 2 * g + j
                    vbf = smp.tile([128, 128], BF16, tag="vbf")
                    nc.vector.tensor_copy(vbf, v[:, t, :])
                    nc.tensor.transpose(vt2[:, j, :], vbf[:], identb_sb[:])
                    nc.tensor.matmul(
                        lt2[:, j, :], v[:, t, :], ablk_sb[:],
                        start=True, stop=True,
                    )
                # xf^T: sum adjacent rrel pairs of VT -> [w, q, c] (bf16)
                vtv = vt2.rearrange("w a (q h c) -> w a q c h", q=4, h=2)
                ce = smp.tile([128, 2, 4, 16], BF16, tag="ce")
                nc.vector.tensor_copy(ce, vtv[:, :, :, :, 0])
                nc.vector.scalar_tensor_tensor(
                    out=xft[:, 2 * g : 2 * g + 2, :, 0:16],
                    in0=vtv[:, :, :, :, 1], scalar=0.0, in1=ce,
                    op0=ALU.bypass, op1=ALU.add,
                )
                # softmax over k (free dim), 0.25-scaled into padded bf16 a
                lb = smp.tile([128, 2, 16], F32, tag="lb")
                nc.vector.tensor_add(
                    lb, lt2,
                    brep_sb[:].unsqueeze(1).broadcast_to((128, 2, 16)))
                lbv = lb.rearrange("w a (q k) -> w a q k", k=4)
                mx = smp.tile([128, 2, 4], F32, tag="mx")
                nc.vector.reduce_max(mx, lbv, axis=AX.X)
                ls = smp.tile([128, 2, 4, 4], F32, tag="ls")
                nc.vector.tensor_sub(
                    ls, lbv, mx.unsqueeze(-1).broadcast_to((128, 2, 4, 4)))
                ae = smp.tile([128, 2, 4, 4], F32, tag="ae")
                nc.scalar.activation(out=ae, in_=ls, func=ACTF.Exp)
                zs = smp.tile([128, 2, 4], F32, tag="zs")
                nc.vector.reduce_sum(zs, ae, axis=AX.X)
                rz = smp.tile([128, 2, 4], F32, tag="rz")
                nc.vector.reciprocal(rz, zs)
                apad = smp.tile([128, 2, 4, 32], BF16, tag="apad")
                nc.vector.scalar_tensor_tensor(
                    out=apad[:, :, :, 0:4], in0=ae, scalar=0.25,
                    in1=rz.unsqueeze(-1).broadcast_to((128, 2, 4, 4)),
                    op0=ALU.mult, op1=ALU.mult,
                )
                for j in range(2):
                    t = 2 * g + j
                    nc.tensor.matmul(
                        g32, apad[:, j, :, :].rearrange("p a b -> p (a b)"),
                        xft[:, t, :, :].rearrange("p a b -> p (a b)"),
                        start=(t == 0), stop=(t == 31),
                    )

            # ================= finale =================
            t0_ = finp.tile([4, 17], F32, tag="t0")
            nc.vector.tensor_copy(t0_, g32[0:4, 0:17])
            t1_ = finp.tile([4, 17], F32, tag="t1")
            nc.vector.tensor_add(t1_, t0_, g32[32:36, 17:34])
            t2_ = finp.tile([4, 17], F32, tag="t2")
            nc.vector.tensor_add(t2_, t1_, g32[64:68, 34:51])
            gsb = finp.tile([4, 17], F32, tag="gsb")
            nc.vector.tensor_add(gsb, t2_, g32[96:100, 51:68])
            cb = finp.tile([4, 16], F32, tag="cb")
            nc.vector.tensor_scalar_mul(cb, cent_sb[:], gsb[:, 16:17])
            v4 = finp.tile([4, 16], F32, tag="v4")
            nc.vector.tensor_sub(v4, gsb[:, 0:16], cb)
            sq = finp.tile([4, 16], F32, tag="sq")
            nc.vector.tensor_mul(sq, v4, v4)
            rs = finp.tile([4, 1], F32, tag="rs")
            nc.vector.reduce_sum(rs, sq, axis=AX.X)
            nrm = finp.tile([4, 1], F32, tag="nrm")
            nc.scalar.activation(out=nrm, in_=rs, func=ACTF.Sqrt)
            nrm2 = finp.tile([4, 1], F32, tag="nrm2")
            nc.vector.tensor_scalar_max(nrm2, nrm, EPS)
            rn = finp.tile([4, 1], F32, tag="rn")
            nc.vector.reciprocal(rn, nrm2)
            vn = finp.tile([4, 16], F32, tag="vn")
            nc.vector.tensor_scalar_mul(vn, v4, rn[:])
            sqn = finp.tile([4, 16], F32, tag="sqn")
            nc.vector.tensor_mul(sqn, vn, vn)
            rs2 = finp.tile([4, 1], F32, tag="rs2")
            nc.vector.reduce_sum(rs2, sqn, axis=AX.X)
            tps = gramp.tile([1, 1], F32, tag="gfin")
            nc.tensor.matmul(tps, ones41_sb[:], rs2[:], start=True, stop=True)
            g1 = finp.tile([1, 1], F32, tag="g1")
            nc.scalar.activation(out=g1, in_=tps, func=ACTF.Sqrt)
            g1m = finp.tile([1, 1], F32, tag="g1m")
            nc.vector.tensor_scalar_max(g1m, g1, EPS)
            g2 = finp.tile([1, 1], F32, tag="g2")
            nc.vector.reciprocal(g2, g1m)
            vcol = finp.tile([64, 1], F32, tag="vcol")
            nc.sync.dma_start(out=vcol, in_=vn[:])
            fps = gramp.tile([1, 7], F32, tag="gfin")
            nc.tensor.matmul(fps, vcol[:], wlin_sb[:], start=True, stop=True)
            osb = finp.tile([1, 7], F32, tag="osb")
            nc.vector.scalar_tensor_tensor(
                out=osb, in0=fps, scalar=g2[:], in1=linb_sb[:],
                op0=ALU.mult, op1=ALU.add,
            )
            nc.sync.dma_start(out=out[img : img + 1, :], in_=osb)

    nc.compile()
    return nc


_CACHE = {}


def kernel(x, conv1_w, conv1_b, conv2_w, conv2_b, centroids, assign_w,
           assign_b, lin_w, lin_b):
    # conv biases are zero in this problem; the banded matrices fold weights
    # only, so assert the assumption the kernel relies on.
    assert np.abs(np.asarray(conv1_b)).max() == 0.0
    assert np.abs(np.asarray(conv2_b)).max() == 0.0

    if "nc" not in _CACHE:
        _CACHE["nc"] = _build_program()
    nc = _CACHE["nc"]

    consts = _build_consts(conv1_w, conv2_w, assign_w, assign_b, lin_w, lin_b)
    # V/xfT are stored as 4x-scaled sums and a is 0.25-scaled, so the
    # centroid term needs abar*4*centroids.
    consts["cent"] = 4.0 * np.asarray(centroids, np.float32)
    xr = _round_f32r(np.asarray(x, np.float32))

    in_maps = []
    for c in range(N_CORES):
        m = dict(consts)
        m["x"] = np.ascontiguousarray(xr[c * IPC : (c + 1) * IPC])
        in_maps.append(m)
    res = run_bass_kernel_spmd(nc, in_maps, list(range(N_CORES))).results
    return np.concatenate([res[c]["out"] for c in range(N_CORES)], axis=0)


if __name__ == "__main__":
    print("smoke test: building program only")
    _build_program()
    print("ok")



# revision 7
# speedup vs baseline: 2.1331x; 2.1331x over previous
"""Trainium2 Bass kernel for conv-stack + NetVLAD pooling + linear head.

Pure data parallel: 32 images sharded 4-per-core across 8 NeuronCores.

v2 design (bf16 data path, f32 PSUM/softmax/finale):
  - input pre-banded on host to [102=(a,ci), 17 tiles, 4 img, 514 w] bf16
    (same layout as SBUF) and loaded with 6 fat chunk DMAs.
  - conv1 (3->4) banded bf16 matmuls; relu+w-pool-sum split ACT/DVE
    -> y1all bf16.
  - conv2 (4->16) banded bf16; odd windows staged with TWO contiguous
    SBUF->SBUF DMAs per image; relu+w-pool split ACT/DVE -> V bf16.
  - NetVLAD: ONE matmul per tile  V_t.T @ [Ppool | 0.25*A]  gives pooled
    xf^T and logits^T together; bulk per-image softmax; unpadded gram
    (lhsT = a [128,16], rhs = xft [128,68]) accumulated over 32 tiles.
  - emission software-pipelined: gram/finale of image i go after conv1 of
    image i+1 so the PE never waits on the softmax chain.
"""
import sys

sys.path.insert(0, "/opt/trn_rl_repo")

import numpy as np
import ml_dtypes
import concourse.bacc as bacc
import concourse.tile as tile
from concourse import mybir
from concourse.bass_utils import run_bass_kernel_spmd

F32 = mybir.dt.float32
BF16 = mybir.dt.bfloat16
AX = mybir.AxisListType
ALU = mybir.AluOpType
ACTF = mybir.ActivationFunctionType

N_CORES = 8
IPC = 4  # images per core
EPS = 1e-12
NBF = ml_dtypes.bfloat16

# input chunk boundaries over conv1 tiles r=0..16 (6 chunks, 2 queues)
CHUNKS = [(0, 3), (3, 6), (6, 9), (9, 12), (12, 15), (15, 17)]


def _build_consts(conv1_w, conv2_w, assign_w, assign_b, lin_w, lin_b):
    c1w = np.asarray(conv1_w, np.float32)
    c2w = np.asarray(conv2_w, np.float32)
    # conv1 banded lhsT, tiles re-anchored to output rows [32r-2, 32r+30):
    # rows p = a*3 + ci (a = h_in - (32r-3), 0..33), cols q = j*4 + co
    # (j = h_out - (32r-2), 0..31). variants: 0 = mid, 1 = first tile
    # (inputs h<0 and outputs h<0 masked), 2 = tail tile r=16 (only
    # outputs 510/511 and inputs 509..511 kept).
    W1 = np.zeros((102, 9, 128), np.float32)
    for dx in range(3):
        for co in range(4):
            for ci in range(3):
                for dy in range(3):
                    for j in range(32):
                        W1[(j + dy) * 3 + ci, dx, j * 4 + co] = c1w[co, ci, dy, dx]
    W1[:, 3:6, :] = W1[:, 0:3, :]
    W1[0:9, 3:6, :] = 0.0        # inputs h=-3..-1
    W1[:, 3:6, 0:8] = 0.0        # outputs h=-2,-1
    W1[:, 6:9, :] = W1[:, 0:3, :]
    W1[9:102, 6:9, :] = 0.0      # inputs h>=512
    W1[:, 6:9, 8:128] = 0.0      # outputs h>=512
    # conv2 banded lhsT with pool1-h fold (input rows are unpooled Y1 rows)
    # and the w-pools stored as SUMs: total scale 0.25.
    W2 = np.zeros((80, 3, 128), np.float32)
    for dx in range(3):
        for co in range(16):
            for ci in range(4):
                for dy in range(3):
                    for rr in range(8):
                        for half in range(2):
                            W2[(2 * rr + 2 * dy + half) * 4 + ci, dx, rr * 16 + co] = (
                                0.25 * c2w[co, ci, dy, dx]
                            )
    # fused pool+assign rhs: rows p = rrel*16 + c.
    # cols 0:64  = Ppool: xft[w, 16q+c] = sum_{rr in 2q,2q+1} V[(rr,c), w]
    # cols 64:80 = 0.25*A: logitsT[w, 4q+k] (pool2-h fold, V stored as sums)
    aw = np.asarray(assign_w, np.float32)
    PA = np.zeros((128, 80), np.float32)
    for q in range(4):
        for half in range(2):
            for c in range(16):
                PA[(2 * q + half) * 16 + c, 16 * q + c] = 1.0
                for k in range(4):
                    PA[(2 * q + half) * 16 + c, 64 + q * 4 + k] = 0.25 * aw[k, c]
    brep = np.tile(np.asarray(assign_b, np.float32), 4).reshape(16)
    brep128 = np.broadcast_to(brep, (128, 16)).copy()
    return {
        "w1": W1.astype(NBF),
        "w2": W2.astype(NBF),
        "pa": PA.astype(NBF),
        "brep": brep128.astype(np.float32),
        "cent": np.zeros(0),  # set by caller (4x centroids)
        "wlin": np.asarray(lin_w, np.float32).T.copy(),  # (64, 7)
        "linb": np.asarray(lin_b, np.float32).reshape(1, 7),
        "ones41": np.ones((4, 1), np.float32),
    }


def _band_input(x4):
    """x4 (4, 3, 512, 512) f32 -> banded bf16 [102, 17, 4, 514]."""
    xpad = np.zeros((546, 3, IPC, 514), np.float32)
    xpad[3:515, :, :, 1:513] = np.asarray(x4, np.float32).transpose(2, 1, 0, 3)
    p = np.arange(102)
    r = np.arange(17)
    idx_h = 32 * r[None, :] + (p // 3)[:, None]          # [102, 17]
    idx_c = np.broadcast_to((p % 3)[:, None], (102, 17))  # [102, 17]
    return np.ascontiguousarray(xpad[idx_h, idx_c]).astype(NBF)


def _build_program():
    nc = bacc.Bacc("TRN2", target_bir_lowering=False, debug=False,
                   num_devices=N_CORES)
    xe = nc.dram_tensor("xe", [102, 17, IPC, 514], BF16, kind="ExternalInput").ap()
    w1 = nc.dram_tensor("w1", [102, 9, 128], BF16, kind="ExternalInput").ap()
    w2 = nc.dram_tensor("w2", [80, 3, 128], BF16, kind="ExternalInput").ap()
    pa = nc.dram_tensor("pa", [128, 80], BF16, kind="ExternalInput").ap()
    brep = nc.dram_tensor("brep", [128, 16], F32, kind="ExternalInput").ap()
    cent = nc.dram_tensor("cent", [4, 16], F32, kind="ExternalInput").ap()
    wlin = nc.dram_tensor("wlin", [64, 7], F32, kind="ExternalInput").ap()
    linb = nc.dram_tensor("linb", [1, 7], F32, kind="ExternalInput").ap()
    ones41 = nc.dram_tensor("ones41", [4, 1], F32, kind="ExternalInput").ap()
    out = nc.dram_tensor("out", [IPC, 7], F32, kind="ExternalOutput").ap()

    from contextlib import ExitStack

    with tile.TileContext(nc) as tc, ExitStack() as es:
        consts = es.enter_context(tc.tile_pool(name="consts", bufs=1))
        big = es.enter_context(tc.tile_pool(name="big", bufs=1))
        x2p = es.enter_context(tc.tile_pool(name="x2p", bufs=2))
        smp = es.enter_context(tc.tile_pool(name="smp", bufs=3))
        xftp = es.enter_context(tc.tile_pool(name="xftp", bufs=8))
        lbp = es.enter_context(tc.tile_pool(name="lbp", bufs=2))
        finp = es.enter_context(tc.tile_pool(name="finp", bufs=2))
        p1p = es.enter_context(tc.tile_pool(name="p1p", bufs=2, space="PSUM"))
        p2p = es.enter_context(tc.tile_pool(name="p2p", bufs=2, space="PSUM"))
        pmp = es.enter_context(tc.tile_pool(name="pmp", bufs=2, space="PSUM"))
        g2p = es.enter_context(tc.tile_pool(name="g2p", bufs=2, space="PSUM"))

        w1_sb = consts.tile([102, 9, 128], BF16)
        nc.sync.dma_start(out=w1_sb, in_=w1)
        w2_sb = consts.tile([80, 3, 128], BF16)
        nc.sync.dma_start(out=w2_sb, in_=w2)
        pa_sb = consts.tile([128, 80], BF16)
        nc.sync.dma_start(out=pa_sb, in_=pa)
        brep_sb = consts.tile([128, 16], F32)
        nc.sync.dma_start(out=brep_sb, in_=brep)
        cent_sb = consts.tile([4, 16], F32)
        nc.sync.dma_start(out=cent_sb, in_=cent)
        wlin_sb = consts.tile([64, 7], F32)
        nc.sync.dma_start(out=wlin_sb, in_=wlin)
        linb_sb = consts.tile([1, 7], F32)
        nc.sync.dma_start(out=linb_sb, in_=linb)
        ones41_sb = consts.tile([4, 1], F32)
        nc.sync.dma_start(out=ones41_sb, in_=ones41)

        # persistent per-image-set buffers
        x1all = big.tile([102, 17, IPC, 514], BF16)
        y1all = big.tile([128, 17, IPC, 258], BF16)
        # vall: tile t = 2*tt + ph  ->  vall[:, img, ph, tt, :]
        vall = big.tile([128, IPC, 2, 16, 128], BF16)
        # double-buffered block-padded softmax weights: cols 0:4 of each
        # 32-block hold a, the rest stay zero so the gram's q-diagonal
        # blocks land on PSUM partitions {0,32,64,96}.
        apadA = big.tile([128, 32, 4, 32], BF16)
        apadB = big.tile([128, 32, 4, 32], BF16)
        apads = [apadA, apadB]
        nc.gpsimd.memset(apadA[:, :, :, 4:32], 0.0)
        nc.gpsimd.memset(apadB[:, :, :, 4:32], 0.0)

        for ic, (r0, r1) in enumerate(CHUNKS):
            eng = nc.sync if ic % 2 == 0 else nc.scalar
            eng.dma_start(out=x1all[:, r0:r1, :, :], in_=xe[:, r0:r1, :, :])

        # conv2 reads y1 cols 0 and 257 (dx shifts); zero them once.
        nc.vector.memset(y1all[:, :, :, 0:1], 0.0)
        nc.vector.memset(y1all[:, :, :, 257:258], 0.0)

        def conv1(img):
            for r in range(17):
                var1 = 1 if r == 0 else (2 if r == 16 else 0)
                p1 = p1p.tile([128, 512], F32, tag="p1")
                for dx in range(3):
                    nc.tensor.matmul(
                        p1, w1_sb[:, var1 * 3 + dx, :],
                        x1all[:, r, img, dx : dx + 512],
                        start=(dx == 0), stop=(dx == 2),
                    )
                p1v = p1.rearrange("p (w two) -> p w two", two=2)
                re1 = smp.tile([128, 256], F32, tag="re1")
                nc.scalar.activation(out=re1, in_=p1v[:, :, 0], func=ACTF.Relu)
                nc.vector.scalar_tensor_tensor(
                    out=y1all[:, r, img, 1:257], in0=p1v[:, :, 1], scalar=0.0,
                    in1=re1, op0=ALU.max, op1=ALU.add,
                )

        def stage(img):
            # odd windows: rows 64:128 of blocks 0..15 and rows 0:16 of
            # blocks 1..16, as two contiguous SBUF->SBUF DMAs.
            x2 = x2p.tile([80, 16, 258], BF16, tag="x2")
            nc.scalar.dma_start(out=x2[0:64, :, :], in_=y1all[64:128, 0:16, img, :])
            nc.scalar.dma_start(out=x2[64:80, :, :], in_=y1all[0:16, 1:17, img, :])
            return x2

        def conv2(img, x2):
            for pi in range(16):
                even = pi < 8
                if even:
                    b = 2 * pi
                    rhs = y1all[0:80, b : b + 2, img, :]
                    ph, tt = 0, 2 * pi
                else:
                    oi = pi - 8
                    rhs = x2[:, 2 * oi : 2 * oi + 2, :]
                    ph, tt = 1, 2 * oi
                p2 = p2p.tile([128, 2, 256], F32, tag="p2")
                for dx in range(3):
                    nc.tensor.matmul(
                        p2, w2_sb[:, dx, :], rhs[:, :, dx : dx + 256],
                        start=(dx == 0), stop=(dx == 2),
                    )
                p2v = p2.rearrange("p a (w two) -> p a w two", two=2)
                re2 = smp.tile([128, 2, 128], F32, tag="re2")
                nc.scalar.activation(out=re2, in_=p2v[:, :, :, 0], func=ACTF.Relu)
                nc.vector.scalar_tensor_tensor(
                    out=vall[:, img, ph, tt : tt + 2, :], in0=p2v[:, :, :, 1],
                    scalar=0.0, in1=re2, op0=ALU.max, op1=ALU.add,
                )

        def poolmm(img):
            # per 4-tile group: xf^T (pooled) + logitsT via one matmul/tile
            lball = lbp.tile([128, 32, 16], F32, tag="lb")
            xfts = []
            for g in range(8):
                pm = pmp.tile([128, 4, 80], F32, tag="pm")
                for j in range(4):
                    t = 4 * g + j
                    nc.tensor.matmul(
                        pm[:, j, :], vall[:, img, t % 2, t // 2, :], pa_sb[:],
                        start=True, stop=True,
                    )
                xft = xftp.tile([128, 4, 4, 17], BF16, tag="xft")
                nc.gpsimd.memset(xft[:, :, :, 16:17], 1.0)
                nc.scalar.copy(
                    xft[:, :, :, 0:16],
                    pm[:, :, 0:64].rearrange("p j (q c) -> p j q c", q=4),
                )
                nc.vector.tensor_add(
                    lball[:, 4 * g : 4 * g + 4, :], pm[:, :, 64:80],
                    brep_sb[:].unsqueeze(1).broadcast_to((128, 4, 16)),
                )
                xfts.append(xft)
            return lball, xfts

        def softmax(img, lball):
            lbv = lball.rearrange("p t (q k) -> p t q k", k=4)
            mx = smp.tile([128, 32, 4], F32, tag="mx")
            nc.vector.reduce_max(mx, lbv, axis=AX.X)
            ls = smp.tile([128, 32, 4, 4], F32, tag="ls")
            nc.vector.tensor_sub(
                ls, lbv, mx.unsqueeze(-1).broadcast_to((128, 32, 4, 4)))
            ae = smp.tile([128, 32, 4, 4], F32, tag="ae")
            nc.scalar.activation(out=ae, in_=ls, func=ACTF.Exp)
            zs = smp.tile([128, 32, 4], F32, tag="zs")
            nc.vector.reduce_sum(zs, ae, axis=AX.X)
            rz = smp.tile([128, 32, 4], F32, tag="rz")
            nc.vector.reciprocal(rz, zs)
            apad = apads[img % 2]
            nc.vector.scalar_tensor_tensor(
                out=apad[:, :, :, 0:4], in0=ae, scalar=0.25,
                in1=rz.unsqueeze(-1).broadcast_to((128, 32, 4, 4)),
                op0=ALU.mult, op1=ALU.mult,
            )
            return apad

        def gram(img, apad, xfts):
            g2 = g2p.tile([128, 68], F32, tag="g2")
            for t in range(32):
                nc.tensor.matmul(
                    g2, apad[:, t, :, :].rearrange("p a b -> p (a b)"),
                    xfts[t // 4][:, t % 4, :, :].rearrange("p a b -> p (a b)"),
                    start=(t == 0), stop=(t == 31),
                )
            return g2

        def finale(img, g2):
            # gsb[k, 0:16] = vlad-sums over clusters' diag blocks;
            # gsb[k, 16] = 0.25 * sum(a)
            t0_ = finp.tile([4, 17], F32, tag="t0")
            nc.vector.tensor_copy(t0_, g2[0:4, 0:17])
            t1_ = finp.tile([4, 17], F32, tag="t1")
            nc.vector.tensor_add(t1_, t0_, g2[32:36, 17:34])
            t2_ = finp.tile([4, 17], F32, tag="t2")
            nc.vector.tensor_add(t2_, t1_, g2[64:68, 34:51])
            gsb = finp.tile([4, 17], F32, tag="gsb")
            nc.vector.tensor_add(gsb, t2_, g2[96:100, 51:68])
            cb = finp.tile([4, 16], F32, tag="cb")
            nc.vector.tensor_scalar_mul(cb, cent_sb[:], gsb[:, 16:17])
            v4 = finp.tile([4, 16], F32, tag="v4")
            nc.vector.tensor_sub(v4, gsb[:, 0:16], cb)
            sq = finp.tile([4, 16], F32, tag="sq")
            nc.vector.tensor_mul(sq, v4, v4)
            rs = finp.tile([4, 1], F32, tag="rs")
            nc.vector.reduce_sum(rs, sq, axis=AX.X)
            nrm = finp.tile([4, 1], F32, tag="nrm")
            nc.scalar.activation(out=nrm, in_=rs, func=ACTF.Sqrt)
            nrm2 = finp.tile([4, 1], F32, tag="nrm2")
            nc.vector.tensor_scalar_max(nrm2, nrm, EPS)
            rn = finp.tile([4, 1], F32, tag="rn")
            nc.vector.reciprocal(rn, nrm2)
            vn = finp.tile([4, 16], F32, tag="vn")
            nc.vector.tensor_scalar_mul(vn, v4, rn[:])
            sqn = finp.tile([4, 16], F32, tag="sqn")
            nc.vector.tensor_mul(sqn, vn, vn)
            rs2 = finp.tile([4, 1], F32, tag="rs2")
            nc.vector.reduce_sum(rs2, sqn, axis=AX.X)
            tps = g2p.tile([1, 1], F32, tag="g2")
            nc.tensor.matmul(tps, ones41_sb[:], rs2[:], start=True, stop=True)
            g1 = finp.tile([1, 1], F32, tag="g1")
            nc.scalar.activation(out=g1, in_=tps, func=ACTF.Sqrt)
            g1m = finp.tile([1, 1], F32, tag="g1m")
            nc.vector.tensor_scalar_max(g1m, g1, EPS)
            g2s = finp.tile([1, 1], F32, tag="g2s")
            nc.vector.reciprocal(g2s, g1m)
            vcol = finp.tile([64, 1], F32, tag="vcol")
            nc.sync.dma_start(out=vcol, in_=vn[:])
            fps = g2p.tile([1, 7], F32, tag="g2")
            nc.tensor.matmul(fps, vcol[:], wlin_sb[:], start=True, stop=True)
            osb = finp.tile([1, 7], F32, tag="osb")
            nc.vector.scalar_tensor_tensor(
                out=osb, in0=fps, scalar=g2s[:], in1=linb_sb[:],
                op0=ALU.mult, op1=ALU.add,
            )
            nc.sync.dma_start(out=out[img : img + 1, :], in_=osb)

        # software-pipelined emission
        pend = None  # (apad, xfts) of previous image awaiting gram+finale
        for img in range(IPC):
            conv1(img)
            if pend is not None:
                pimg, papad, pxfts = pend
                finale(pimg, gram(pimg, papad, pxfts))
            x2 = stage(img)
            conv2(img, x2)
            lball, xfts = poolmm(img)
            apad = softmax(img, lball)
            pend = (img, apad, xfts)
        pimg, papad, pxfts = pend
        finale(pimg, gram(pimg, papad, pxfts))

    nc.compile()
    return nc


_CACHE = {}


def kernel(x, conv1_w, conv1_b, conv2_w, conv2_b, centroids, assign_w,
           assign_b, lin_w, lin_b):
    # conv biases are zero in this problem; the banded matrices fold weights
    # only, so assert the assumption the kernel relies on.
    assert np.abs(np.asarray(conv1_b)).max() == 0.0
    assert np.abs(np.asarray(conv2_b)).max() == 0.0

    if "nc" not in _CACHE:
        _CACHE["nc"] = _build_program()
    nc = _CACHE["nc"]

    consts = _build_consts(conv1_w, conv2_w, assign_w, assign_b, lin_w, lin_b)
    # V/xfT are stored as 4x-scaled sums and a is 0.25-scaled, so the
    # centroid term needs asum*4*centroids.
    consts["cent"] = 4.0 * np.asarray(centroids, np.float32)

    x = np.asarray(x, np.float32)
    in_maps = []
    for c in range(N_CORES):
        m = dict(consts)
        m["xe"] = _band_input(x[c * IPC : (c + 1) * IPC])
        in_maps.append(m)
    res = run_bass_kernel_spmd(nc, in_maps, list(range(N_CORES))).results
    return np.concatenate([res[c]["out"] for c in range(N_CORES)], axis=0)


if __name__ == "__main__":
    print("smoke test: building program only")
    _build_program()
    print("ok")
